# revision 1
# baseline (speedup 1.0000x reference)
"""Deep Neural Decision Forest kernel for 8x Trainium2 NeuronCores.

Strategy: data-parallel over batch (4096 -> 8 x 512). Each core runs an
identical Bass/Tile program over its batch shard with batch on the matmul
free (N) dimension throughout ("transposed" layouts, feature dims on
partitions), so no on-device transposes are needed:

  conv1 (Toeplitz matmul) -> relu+maxpool (DVE) -> conv2 (Toeplitz matmul)
  -> relu+maxpool -> per-tree MLP (matmul) -> routing in log space:
     logmu^T = (w2 A)^T th - P^T softplus(z),  mu = exp(logmu)
  -> py^T = sum_t leafp_scaled[t]^T mu_t  (PSUM accumulation)
  -> out = ln(py^T)

where A[n,l] = 1 if leaf l goes left at node n, P[n,l] = 1 if node n is on
leaf l's path.  log sigmoid(z) = z - softplus(z), log(1-sigmoid(z)) =
-softplus(z) turn the depth-product into matmuls; softplus = ln(1+exp(z))
keeps all ACT work in the single natural_log_exp table set.
"""

import numpy as np
import ml_dtypes

import concourse.bass as bass
import concourse.tile as tile
from concourse import bacc, mybir
from concourse.alu_op_type import AluOpType
from concourse.bass_utils import run_bass_kernel_spmd

AF = mybir.ActivationFunctionType
F32 = mybir.dt.float32
BF16 = mybir.dt.bfloat16

NDEPTH, NLABEL, NTREE, B = 6, 10, 32, 4096
NLEAF = 128
NCORES = 8
BC = B // NCORES  # 512 batch per core

BF = ml_dtypes.bfloat16


def _patch_act_tables():
    """Make Exp/Ln resolvable only via natural_log_exp_and_others so the
    table-load inserter cannot ping-pong between the exp-only and ln-only
    sets (each switch costs ~2.7us on ACT). Set positions are preserved."""
    if getattr(bacc, "_ddf_act_patch", False):
        return
    import concourse.hw_specs as hs
    orig = hs.get_activation_tables

    def patched(module_arch):
        tabs = orig(module_arch)
        for name, funcs in tabs.items():
            if name != "natural_log_exp_and_others":
                funcs.discard(AF.Exp)
                funcs.discard(AF.Ln)
        return tabs

    bacc.get_activation_tables = patched
    bacc._ddf_act_patch = True


# ---------------------------------------------------------------- host math
def _routing():
    node = np.zeros((NDEPTH + 1, NLEAF), np.int32)
    left = np.zeros((NDEPTH + 1, NLEAF), bool)
    left[0] = np.arange(NLEAF) < NLEAF // 2
    for d in range(1, NDEPTH + 1):
        w = 2 ** (NDEPTH - d + 1)
        j = np.arange(NLEAF)
        node[d] = 2**d - 1 + j // w
        left[d] = (j % w) < w // 2
    return node, left


def _route_mats():
    node, left = _routing()
    A = np.zeros((128, 128), np.float32)
    P = np.zeros((128, 128), np.float32)
    for d in range(NDEPTH + 1):
        for l in range(NLEAF):
            n = node[d, l]
            P[n, l] = 1.0
            if left[d, l]:
                A[n, l] = 1.0
    return A, P


def _conv1_toeplitz(w1c):
    # T1[q] [112,120]: rows r=(ky,px) ky 0..3 px 0..27; cols c=(oc,i) ox=2i+q
    # T2[q] [112,120]: rows r=px for ky=4 (r<28), zero beyond
    t1 = np.zeros((2, 112, 120), np.float32)
    t2 = np.zeros((2, 112, 120), np.float32)
    for q in range(2):
        for oc in range(10):
            for i in range(12):
                ox = 2 * i + q
                c = oc * 12 + i
                for kx in range(5):
                    px = ox + kx
                    for ky in range(4):
                        t1[q, 28 * ky + px, c] = w1c[oc, 0, ky, kx]
                    t2[q, px, c] = w1c[oc, 0, 4, kx]
    return t1, t2


def _conv2_toeplitz(w2c):
    # W2T[ky,q] [120,80]: rows r=(ic,px) px 0..11; cols c=(oc,i) ox=2i+q
    t = np.zeros((5, 2, 120, 80), np.float32)
    for ky in range(5):
        for q in range(2):
            for oc in range(20):
                for i in range(4):
                    ox = 2 * i + q
                    c = oc * 4 + i
                    for kx in range(5):
                        px = ox + kx
                        for ic in range(10):
                            t[ky, q, ic * 12 + px, c] = w2c[oc, ic, ky, kx]
    return t


def _precompute(inputs):
    """Host-side derived weights (numpy float32).

    Per-tree weight matrices are packed side-by-side in the free dimension
    into a few large tensors so each loads in one big contiguous DMA."""
    x = np.asarray(inputs["x"], np.float32).reshape(B, 784)
    w1c = np.asarray(inputs["conv1_w"], np.float32)
    b1c = np.asarray(inputs["conv1_b"], np.float32)
    w2c = np.asarray(inputs["conv2_w"], np.float32)
    b2c = np.asarray(inputs["conv2_b"], np.float32)
    w1 = np.asarray(inputs["w1"], np.float32)   # [T,320,50]
    b1 = np.asarray(inputs["b1"], np.float32)   # [T,50]
    w2 = np.asarray(inputs["w2"], np.float32)   # [T,50,128]
    b2 = np.asarray(inputs["b2"], np.float32)   # [T,128]
    pi = np.asarray(inputs["pi"], np.float32)   # [T,128,10]

    assert np.all(b1c == 0) and np.all(b2c == 0), "conv biases assumed zero"
    assert np.all(b1 == 0) and np.all(b2 == 0), "mlp biases assumed zero"

    A, P = _route_mats()

    t1, t2 = _conv1_toeplitz(w1c)
    # tq [112, 480]: (T1_q0 | T2_q0 | T1_q1 | T2_q1)
    tq = np.concatenate([t1[0], t2[0], t1[1], t2[1]], axis=1)

    w2t5 = _conv2_toeplitz(w2c)
    # w2tall [120, 800]: col block (ky*2+q)*80
    w2tall = np.zeros((120, 800), np.float32)
    for ky in range(5):
        for q in range(2):
            b_ = (ky * 2 + q) * 80
            w2tall[:, b_:b_ + 80] = w2t5[ky, q]

    # w1pall [80, 64*128]: block (j*4+y)*128; tree 2j at +0:50, 2j+1 at +64:114
    w1pall = np.zeros((80, 64 * 128), np.float32)
    for j in range(16):
        for y in range(4):
            f_idx = (np.arange(20)[:, None] * 16 + y * 4
                     + np.arange(4)[None, :]).reshape(80)
            blk = (j * 4 + y) * 128
            w1pall[:, blk:blk + 50] = w1[2 * j][f_idx]
            w1pall[:, blk + 64:blk + 114] = w1[2 * j + 1][f_idx]

    # w2all / w2aall [128, 32*128]: tree t at cols t*128, th rows (t%2)*64..
    w2all = np.zeros((128, 32 * 128), np.float32)
    w2aall = np.zeros((128, 32 * 128), np.float32)
    for t in range(32):
        s = t % 2
        w2all[s * 64:s * 64 + 50, t * 128:(t + 1) * 128] = w2[t]
        w2a = w2[t][:, :127] @ A[:127, :]
        w2aall[s * 64:s * 64 + 50, t * 128:(t + 1) * 128] = w2a

    negp = -P  # [128,128]

    pim = pi - pi.max(axis=-1, keepdims=True)
    e = np.exp(pim)
    leafp = e / e.sum(axis=-1, keepdims=True)
    leafp_s = leafp / float(NLEAF * NTREE)
    lpall = np.zeros((128, 32 * 128), np.float32)
    for t in range(32):
        lpall[:, t * 128:t * 128 + 10] = leafp_s[t]

    # input: XT padded [896, B] pixel-major, zeros past 783
    xt = np.zeros((896, B), np.float32)
    xt[:784] = x.T

    return dict(xt=xt, tq=tq, w2tall=w2tall, w1pall=w1pall, w2all=w2all,
                w2aall=w2aall, negp=negp, lpall=lpall)


# ------------------------------------------------------------- bass program
def _build_nc(n_loop=1):
    _patch_act_tables()
    nc = bacc.Bacc("TRN2", target_bir_lowering=False, debug=False,
                   num_devices=NCORES)

    d_xt = nc.dram_tensor("xt", [896, BC], BF16, kind="ExternalInput").ap()
    d_tq = nc.dram_tensor("tq", [112, 480], BF16, kind="ExternalInput").ap()
    d_w2t = nc.dram_tensor("w2tall", [120, 800], BF16, kind="ExternalInput").ap()
    d_w1p = nc.dram_tensor("w1pall", [80, 64 * 128], BF16,
                           kind="ExternalInput").ap()
    d_w2all = nc.dram_tensor("w2all", [128, 32 * 128], BF16,
                             kind="ExternalInput").ap()
    d_w2aall = nc.dram_tensor("w2aall", [128, 32 * 128], BF16,
                              kind="ExternalInput").ap()
    d_negp = nc.dram_tensor("negp", [128, 128], BF16, kind="ExternalInput").ap()
    d_lpall = nc.dram_tensor("lpall", [128, 32 * 128], BF16,
                             kind="ExternalInput").ap()
    d_out = nc.dram_tensor("out", [10, BC], F32, kind="ExternalOutput").ap()

    with tile.TileContext(nc) as tc:
        _emit(tc, d_xt, d_tq, d_w2t, d_w1p, d_w2all, d_w2aall,
              d_negp, d_lpall, d_out, n_loop=n_loop)
    nc.compile()
    return nc


def _emit(tc, d_xt, d_tq, d_w2t, d_w1p, d_w2all, d_w2aall,
          d_negp, d_lpall, d_out, n_loop=1):
    from contextlib import ExitStack
    nc = tc.nc
    ctx = ExitStack()
    with ctx:
        consts = ctx.enter_context(tc.tile_pool(name="consts", bufs=1))
        work = ctx.enter_context(tc.tile_pool(name="work", bufs=1))
        tmp = ctx.enter_context(tc.tile_pool(name="tmp", bufs=4))
        ps = ctx.enter_context(tc.tile_pool(name="ps", bufs=7, space="PSUM"))
        pyp = ctx.enter_context(tc.tile_pool(name="pyp", bufs=1, space="PSUM"))

        # ---- load constants (few big DMAs)
        xm = {}
        for m in range(4):
            for k in range(7):
                t = consts.tile([112, BC], BF16, tag=f"xm{m}_{k}")
                nc.sync.dma_start(
                    out=t[:],
                    in_=d_xt[28 * m + 112 * k:28 * m + 112 * k + 112, :])
                xm[(m, k)] = t
        tq = consts.tile([112, 480], BF16, tag="tq")
        nc.sync.dma_start(out=tq[:], in_=d_tq)
        w2t = consts.tile([120, 800], BF16, tag="w2t")
        nc.sync.dma_start(out=w2t[:], in_=d_w2t)
        w1p = consts.tile([80, 64 * 128], BF16, tag="w1p")
        nc.sync.dma_start(out=w1p[:], in_=d_w1p)
        w2all = consts.tile([128, 32 * 128], BF16, tag="w2all")
        nc.sync.dma_start(out=w2all[:], in_=d_w2all)
        w2aall = consts.tile([128, 32 * 128], BF16, tag="w2aall")
        nc.sync.dma_start(out=w2aall[:], in_=d_w2aall)
        negp = consts.tile([128, 128], BF16, tag="negp")
        nc.sync.dma_start(out=negp[:], in_=d_negp)
        lpall = consts.tile([128, 32 * 128], BF16, tag="lpall")
        nc.sync.dma_start(out=lpall[:], in_=d_lpall)

        def _compute():
            # ---- conv1 + pool -> H1_r [120, BC] bf16, r = 0..11
            h1 = {}
            for r in range(12):
                aps = {}
                for dy in range(2):
                    oy = 2 * r + dy
                    m, k = oy % 4, oy // 4
                    for q in range(2):
                        p = ps.tile([128, BC], F32, tag="ps")
                        nc.tensor.matmul(out=p[:120, :],
                                         lhsT=tq[:, q * 240:q * 240 + 120],
                                         rhs=xm[(m, k)][:], start=True, stop=False)
                        nc.tensor.matmul(out=p[:120, :],
                                         lhsT=tq[:, q * 240 + 120:q * 240 + 240],
                                         rhs=xm[(m, k + 1)][:],
                                         start=False, stop=True)
                        aps[(dy, q)] = p
                # pool: H1_r = max(0, A00, A01, A10, A11) — chain, 1 psum read/op
                a0 = tmp.tile([120, BC], BF16, tag="mx")
                nc.vector.tensor_scalar_max(a0[:], aps[(0, 0)][:120, :], 0.0)
                a1 = tmp.tile([120, BC], BF16, tag="mx")
                nc.vector.tensor_max(a1[:], aps[(0, 1)][:120, :], a0[:])
                a2 = tmp.tile([120, BC], BF16, tag="mx")
                nc.vector.tensor_max(a2[:], aps[(1, 0)][:120, :], a1[:])
                h = work.tile([120, BC], BF16, tag=f"h1_{r}")
                nc.vector.tensor_max(h[:], aps[(1, 1)][:120, :], a2[:])
                h1[r] = h

            # ---- conv2 + pool -> F_y [80, BC] bf16, y = 0..3
            fy = {}
            for y in range(4):
                cps = {}
                for dy in range(2):
                    oy = 2 * y + dy
                    for q in range(2):
                        p = ps.tile([128, BC], F32, tag="ps")
                        for ky in range(5):
                            blk = (ky * 2 + q) * 80
                            nc.tensor.matmul(out=p[:80, :],
                                             lhsT=w2t[:, blk:blk + 80],
                                             rhs=h1[oy + ky][:],
                                             start=(ky == 0), stop=(ky == 4))
                        cps[(dy, q)] = p
                a0 = tmp.tile([80, BC], BF16, tag="mx2")
                nc.vector.tensor_scalar_max(a0[:], cps[(0, 0)][:80, :], 0.0)
                a1 = tmp.tile([80, BC], BF16, tag="mx2")
                nc.vector.tensor_max(a1[:], cps[(0, 1)][:80, :], a0[:])
                a2 = tmp.tile([80, BC], BF16, tag="mx2")
                nc.vector.tensor_max(a2[:], cps[(1, 0)][:80, :], a1[:])
                f = work.tile([80, BC], BF16, tag=f"fy_{y}")
                nc.vector.tensor_max(f[:], cps[(1, 1)][:80, :], a2[:])
                fy[y] = f

            # ---- stage C: TH_j [128, BC] bf16 (tree 2j @ rows 0:50, 2j+1 @ 64:114)
            th = {}
            for j in range(16):
                p = ps.tile([128, BC], F32, tag="ps")
                for y in range(4):
                    blk = (j * 4 + y) * 128
                    nc.tensor.matmul(out=p[:], lhsT=w1p[:, blk:blk + 128],
                                     rhs=fy[y][:], start=(y == 0), stop=(y == 3))
                t = work.tile([128, BC], BF16, tag=f"th_{j}")
                nc.vector.tensor_scalar_max(t[:], p[:], 0.0)
                th[j] = t

            # ---- per tree pair: z, s = ln(1+exp(z)), logmu, mu, py accumulation.
            # K=50 matmuls (z, w2a) use 64-row tiling: the pair's th rows sit at
            # 0:50 / 64:114, so the two matmuls run on different PE row-groups
            # concurrently (outputs in different PSUM banks). negp/lp matmuls are
            # full 128-mode; batching the pair keeps mode switches to 2 per pair.
            py = pyp.tile([128, BC], F32, tag="py")
            for j in range(16):
                pz0 = ps.tile([128, BC], F32, tag="ps")
                pz1 = ps.tile([128, BC], F32, tag="ps")
                for s_, pz in ((0, pz0), (1, pz1)):
                    t_ = 2 * j + s_
                    c0 = t_ * 128
                    r0 = s_ * 64
                    nc.tensor.matmul(out=pz[:],
                                     lhsT=w2all[r0:r0 + 50, c0:c0 + 128],
                                     rhs=th[j][r0:r0 + 50, :],
                                     start=True, stop=True)
                ss = []
                for s_, pz in ((0, pz0), (1, pz1)):
                    e = tmp.tile([128, BC], BF16, tag="e")
                    nc.scalar.activation(out=e[:], in_=pz[:], func=AF.Exp,
                                         bias=0.0, scale=1.0)
                    s = tmp.tile([128, BC], BF16, tag="s")
                    nc.scalar.activation(out=s[:], in_=e[:], func=AF.Ln,
                                         bias=1.0, scale=1.0)
                    ss.append(s)
                p0 = ps.tile([128, BC], F32, tag="ps")
                p1 = ps.tile([128, BC], F32, tag="ps")
                for s_, p in ((0, p0), (1, p1)):
                    t_ = 2 * j + s_
                    c0 = t_ * 128
                    r0 = s_ * 64
                    nc.tensor.matmul(out=p[:],
                                     lhsT=w2aall[r0:r0 + 50, c0:c0 + 128],
                                     rhs=th[j][r0:r0 + 50, :],
                                     start=True, stop=False)
                for s_, p in ((0, p0), (1, p1)):
                    nc.tensor.matmul(out=p[:], lhsT=negp[:], rhs=ss[s_][:],
                                     start=False, stop=True)
                for s_, p in ((0, p0), (1, p1)):
                    t_ = 2 * j + s_
                    c0 = t_ * 128
                    mu = tmp.tile([128, BC], BF16, tag="mu")
                    nc.scalar.activation(out=mu[:], in_=p[:], func=AF.Exp,
                                         bias=0.0, scale=1.0)
                    nc.tensor.matmul(out=py[:], lhsT=lpall[:, c0:c0 + 128],
                                     rhs=mu[:],
                                     start=(t_ == 0), stop=(t_ == 31),
                                     skip_group_check=True)

            out_t = work.tile([10, BC], F32, tag="out")
            nc.scalar.activation(out=out_t[:], in_=py[:10, :], func=AF.Ln)
            nc.sync.dma_start(out=d_out, in_=out_t[:])

        if n_loop == 1:
            _compute()
        else:
            with tc.For_i(0, n_loop, 1):
                _compute()


_NC_CACHE = None


def _get_nc():
    global _NC_CACHE
    if _NC_CACHE is None:
        _NC_CACHE = _build_nc()
    return _NC_CACHE


def make_in_maps(inputs):
    pre = _precompute(inputs)
    shared = {k: pre[k].astype(BF) for k in
              ("tq", "w2tall", "w1pall", "w2all", "w2aall", "negp", "lpall")}
    in_maps = []
    for c in range(NCORES):
        m = dict(shared)
        m["xt"] = np.ascontiguousarray(
            pre["xt"][:, c * BC:(c + 1) * BC]).astype(BF)
        in_maps.append(m)
    return in_maps


def kernel(**inputs):
    nc = _get_nc()
    in_maps = make_in_maps(inputs)
    res = run_bass_kernel_spmd(nc, in_maps, core_ids=list(range(NCORES)))
    outs = [res.results[c]["out"] for c in range(NCORES)]  # each [10, BC]
    full = np.concatenate(outs, axis=1)  # [10, B]
    return np.ascontiguousarray(full.T).astype(np.float32)  # [B, 10]



# revision 7
# speedup vs baseline: 9.4524x; 9.4524x over previous
"""Deep Neural Decision Forest kernel for 8x Trainium2 NeuronCores.

Strategy: data-parallel over batch (4096 -> 8 x 512), batch on the matmul
free (N) dimension throughout. v2: conv1/conv2/tree-MLP matmuls run in
fp8(e4m3) with DoubleRow perf mode (2 K-subtiles per pass -> 2x PE
throughput, half the matmul count); maxpool chains use two-PSUM-read
tensor_max on DVE plus a fused relu-max scalar_tensor_tensor on the (idle)
GpSimd engine; the routing stage is phase-split (all z -> softplus first,
then all routing+exp) so the single natural_log_exp ACT table set loads
once.

  conv1 (Toeplitz fp8 DR) -> pool -> conv2 (fp8 DR) -> pool
  -> per-tree MLP (fp8 DR) -> routing in log space:
     logmu^T = (w2 A)^T th - P^T softplus(z),  mu = exp(logmu)
  -> py^T = sum_t leafp_scaled[t]^T mu_t  (PSUM accumulation)
  -> out = ln(py^T)

fp8 scale chain: weights of conv1/conv2/mlp1 are scaled x8 so their values
sit in e4m3's normal range; activations then carry 8x / 64x / 512x scales
through PSUM, absorbed for free: h1=8x, fy=64x in fp8 (well inside e4m3
range), and th rescales by 1/512 inside its fused relu (dual-op
tensor_scalar). Stage D (z / routing / leaf mixing) stays bf16.
"""

import numpy as np
import ml_dtypes

import concourse.bass as bass
import concourse.tile as tile
from concourse import bacc, mybir
from concourse.alu_op_type import AluOpType
from concourse.bass_utils import run_bass_kernel_spmd

AF = mybir.ActivationFunctionType
F32 = mybir.dt.float32
BF16 = mybir.dt.bfloat16
F8 = mybir.dt.float8e4
DR = mybir.MatmulPerfMode.DoubleRow

NDEPTH, NLABEL, NTREE, B = 6, 10, 32, 4096
NLEAF = 128
NCORES = 8
BC = B // NCORES  # 512 batch per core

BF = ml_dtypes.bfloat16
F8NP = ml_dtypes.float8_e4m3

WS = 8.0  # fp8 weight scale


def _patch_act_tables():
    """Make Exp/Ln resolvable only via natural_log_exp_and_others so the
    table-load inserter cannot ping-pong between the exp-only and ln-only
    sets (each switch costs ~2.7us on ACT). Set positions are preserved."""
    if getattr(bacc, "_ddf_act_patch", False):
        return
    import concourse.hw_specs as hs
    orig = hs.get_activation_tables

    def patched(module_arch):
        tabs = orig(module_arch)
        for name, funcs in tabs.items():
            if name != "natural_log_exp_and_others":
                funcs.discard(AF.Exp)
                funcs.discard(AF.Ln)
        return tabs

    bacc.get_activation_tables = patched
    bacc._ddf_act_patch = True


# ---------------------------------------------------------------- host math
def _routing():
    node = np.zeros((NDEPTH + 1, NLEAF), np.int32)
    left = np.zeros((NDEPTH + 1, NLEAF), bool)
    left[0] = np.arange(NLEAF) < NLEAF // 2
    for d in range(1, NDEPTH + 1):
        w = 2 ** (NDEPTH - d + 1)
        j = np.arange(NLEAF)
        node[d] = 2**d - 1 + j // w
        left[d] = (j % w) < w // 2
    return node, left


def _route_mats():
    node, left = _routing()
    A = np.zeros((128, 128), np.float32)
    P = np.zeros((128, 128), np.float32)
    for d in range(NDEPTH + 1):
        for l in range(NLEAF):
            n = node[d, l]
            P[n, l] = 1.0
            if left[d, l]:
                A[n, l] = 1.0
    return A, P


def _conv1_dr(w1c):
    """tq8 [112, 8 variants (q*4+oy%4), 2 slots, 120]: DoubleRow Toeplitz.
    k_eff = r*112+p covers pixels 112*(oy//4)+k_eff; weight row k_rel =
    k_eff - 28*(oy%4) = 28*ky + ox + kx."""
    t = np.zeros((112, 8, 2, 128), np.float32)  # M padded 120->128 (DoubleRow needs M%16==0)
    for q in range(2):
        for dmod in range(4):
            v = q * 4 + dmod
            for oc in range(10):
                for i in range(12):
                    ox = 2 * i + q
                    m = oc * 12 + i
                    for ky in range(5):
                        for kx in range(5):
                            k_eff = 28 * dmod + 28 * ky + ox + kx
                            r, p = divmod(k_eff, 112)
                            t[p, v, r, m] = WS * w1c[oc, 0, ky, kx]
    return t


def _conv2_dr(w2c):
    """w2t8 [120, 6 variants (q*3+kp), 2, 80]: rows p=(ic,xin), ky=2*kp+r."""
    t = np.zeros((120, 6, 2, 80), np.float32)
    for q in range(2):
        for kp in range(3):
            v = q * 3 + kp
            for oc in range(20):
                for i in range(4):
                    ox = 2 * i + q
                    m = oc * 4 + i
                    for r in range(2):
                        ky = 2 * kp + r
                        if ky >= 5:
                            continue
                        for kx in range(5):
                            xin = ox + kx
                            for ic in range(10):
                                t[ic * 12 + xin, v, r, m] = WS * w2c[oc, ic, ky, kx]
    return t


def _w1p_dr(w1):
    """w1p8 [80, 16 j, 2 half, 2 slot, 128]: p=(ch,xx), y=2*half+slot,
    f = ch*16+y*4+xx; tree 2j at cols 0:50, 2j+1 at 64:114."""
    t = np.zeros((80, 16, 2, 2, 128), np.float32)
    ch = np.arange(20)[:, None]
    xx = np.arange(4)[None, :]
    for j in range(16):
        for h in range(2):
            for r in range(2):
                y = 2 * h + r
                f_idx = (ch * 16 + y * 4 + xx).reshape(80)
                t[:, j, h, r, :50] = WS * w1[2 * j][f_idx]
                t[:, j, h, r, 64:114] = WS * w1[2 * j + 1][f_idx]
    return t


def _precompute(inputs):
    """Host-side derived weights."""
    x = np.asarray(inputs["x"], np.float32).reshape(B, 784)
    w1c = np.asarray(inputs["conv1_w"], np.float32)
    b1c = np.asarray(inputs["conv1_b"], np.float32)
    w2c = np.asarray(inputs["conv2_w"], np.float32)
    b2c = np.asarray(inputs["conv2_b"], np.float32)
    w1 = np.asarray(inputs["w1"], np.float32)   # [T,320,50]
    b1 = np.asarray(inputs["b1"], np.float32)   # [T,50]
    w2 = np.asarray(inputs["w2"], np.float32)   # [T,50,128]
    b2 = np.asarray(inputs["b2"], np.float32)   # [T,128]
    pi = np.asarray(inputs["pi"], np.float32)   # [T,128,10]

    assert np.all(b1c == 0) and np.all(b2c == 0), "conv biases assumed zero"
    assert np.all(b1 == 0) and np.all(b2 == 0), "mlp biases assumed zero"

    A, P = _route_mats()

    tq8 = _conv1_dr(w1c)
    w2t8 = _conv2_dr(w2c)
    w1p8 = _w1p_dr(w1)

    # w2all / w2aall [128, 32*128] bf16: tree t at cols t*128, th rows (t%2)*64..
    w2all = np.zeros((128, 32 * 128), np.float32)
    w2aall = np.zeros((128, 32 * 128), np.float32)
    for t in range(32):
        s = t % 2
        w2all[s * 64:s * 64 + 50, t * 128:(t + 1) * 128] = w2[t]
        w2a = w2[t][:, :127] @ A[:127, :]
        w2aall[s * 64:s * 64 + 50, t * 128:(t + 1) * 128] = w2a

    negp = -P  # [128,128]

    pim = pi - pi.max(axis=-1, keepdims=True)
    e = np.exp(pim)
    leafp = e / e.sum(axis=-1, keepdims=True)
    leafp_s = leafp / float(NLEAF * NTREE)
    lpall = np.zeros((128, 32 * 128), np.float32)
    for t in range(32):
        lpall[:, t * 128:t * 128 + 10] = leafp_s[t]

    # input: interleaved pixel chunks [112, 7, B]: xt[p, c, b] = x[b, 112c+p]
    xt = np.zeros((112, 7, B), np.float32)
    xp = x.T  # [784, B]
    for c in range(7):
        xt[:, c, :] = xp[112 * c:112 * c + 112]

    return dict(xt=xt, tq8=tq8, w2t8=w2t8, w1p8=w1p8, w2all=w2all,
                w2aall=w2aall, negp=negp, lpall=lpall)


# ------------------------------------------------------------- bass program
def _build_nc(n_loop=1):
    _patch_act_tables()
    nc = bacc.Bacc("TRN2", target_bir_lowering=False, debug=False,
                   num_devices=NCORES)

    d_xt = nc.dram_tensor("xt", [112, 7, BC], F8, kind="ExternalInput").ap()
    d_tq8 = nc.dram_tensor("tq8", [112, 16, 128], F8, kind="ExternalInput").ap()
    d_w2t8 = nc.dram_tensor("w2t8", [120, 12, 80], F8, kind="ExternalInput").ap()
    d_w1p8 = nc.dram_tensor("w1p8", [80, 64, 128], F8, kind="ExternalInput").ap()
    d_w2all = nc.dram_tensor("w2all", [128, 32 * 128], BF16,
                             kind="ExternalInput").ap()
    d_w2aall = nc.dram_tensor("w2aall", [128, 32 * 128], BF16,
                              kind="ExternalInput").ap()
    d_negp = nc.dram_tensor("negp", [128, 128], BF16, kind="ExternalInput").ap()
    d_lpall = nc.dram_tensor("lpall", [128, 32 * 128], BF16,
                             kind="ExternalInput").ap()
    d_out = nc.dram_tensor("out", [10, BC], F32, kind="ExternalOutput").ap()

    with tile.TileContext(nc) as tc:
        _emit(tc, d_xt, d_tq8, d_w2t8, d_w1p8, d_w2all, d_w2aall,
              d_negp, d_lpall, d_out, n_loop=n_loop)
    nc.compile()
    return nc


def _emit(tc, d_xt, d_tq8, d_w2t8, d_w1p8, d_w2all, d_w2aall,
          d_negp, d_lpall, d_out, n_loop=1):
    from contextlib import ExitStack
    nc = tc.nc
    ctx = ExitStack()
    with ctx:
        consts = ctx.enter_context(tc.tile_pool(name="consts", bufs=1))
        work = ctx.enter_context(tc.tile_pool(name="work", bufs=1))
        tmp = ctx.enter_context(tc.tile_pool(name="tmp", bufs=4))
        ps = ctx.enter_context(tc.tile_pool(name="ps", bufs=7, space="PSUM"))
        pyp = ctx.enter_context(tc.tile_pool(name="pyp", bufs=1, space="PSUM"))

        # ---- load constants, in first-use order
        tq8 = consts.tile([112, 16, 128], F8, tag="tq8")
        nc.sync.dma_start(out=tq8[:], in_=d_tq8)
        xt = consts.tile([112, 7, BC], F8, tag="xt")
        nc.sync.dma_start(out=xt[:], in_=d_xt)
        w2t8 = consts.tile([120, 12, 80], F8, tag="w2t8")
        nc.sync.dma_start(out=w2t8[:], in_=d_w2t8)
        w1p8 = consts.tile([80, 64, 128], F8, tag="w1p8")
        nc.sync.dma_start(out=w1p8[:], in_=d_w1p8)
        w2all = consts.tile([128, 32 * 128], BF16, tag="w2all")
        nc.sync.dma_start(out=w2all[:], in_=d_w2all)
        w2aall = consts.tile([128, 32 * 128], BF16, tag="w2aall")
        nc.sync.dma_start(out=w2aall[:], in_=d_w2aall)
        negp = consts.tile([128, 128], BF16, tag="negp")
        nc.sync.dma_start(out=negp[:], in_=d_negp)
        lpall = consts.tile([128, 32 * 128], BF16, tag="lpall")
        nc.sync.dma_start(out=lpall[:], in_=d_lpall)

        h1all = work.tile([120, 13, BC], F8, tag="h1all")
        fyall = work.tile([80, 4, BC], F8, tag="fyall")

        def _compute():
            # zero pad chunk read by conv2's (ky=4, ky=5-pad) DoubleRow pass
            nc.gpsimd.memset(h1all[:, 12, :], 0.0)

            # ---- conv1 + pool -> h1all[:, r, :] = 8*h1_true, fp8, r=0..11
            for r in range(12):
                aps = {}
                for dy in range(2):
                    oy = 2 * r + dy
                    k, dmod = oy // 4, oy % 4
                    for q in range(2):
                        v = q * 4 + dmod
                        p = ps.tile([128, BC], F32, tag="ps")
                        nc.tensor.matmul(out=p[:],
                                         lhsT=tq8[:, 2 * v:2 * v + 2, :],
                                         rhs=xt[:, k:k + 2, :],
                                         start=True, stop=True, perf_mode=DR)
                        aps[(dy, q)] = p
                # 4-way max + relu: walrus allows ONE PSUM operand per op, so
                # ACT relu-copies two accumulators (idle during conv) and DVE
                # folds the other two; b0/b1 >= 0 so the final max is the relu.
                a0 = tmp.tile([120, BC], BF16, tag="mx")
                nc.scalar.activation(out=a0[:], in_=aps[(0, 0)][:120, :],
                                     func=AF.Relu)
                a1 = tmp.tile([120, BC], BF16, tag="mx")
                nc.scalar.activation(out=a1[:], in_=aps[(0, 1)][:120, :],
                                     func=AF.Relu)
                b0 = tmp.tile([120, BC], BF16, tag="mx")
                nc.vector.tensor_max(b0[:], aps[(1, 0)][:120, :], a0[:])
                b1 = tmp.tile([120, BC], BF16, tag="mx")
                nc.vector.tensor_max(b1[:], aps[(1, 1)][:120, :], a1[:])
                nc.vector.tensor_max(h1all[:, r, :], b0[:], b1[:])

            # ---- conv2 + pool -> fyall[:, y, :] = 64*fy_true, fp8, y=0..3
            for y in range(4):
                cps = {}
                for dy in range(2):
                    oy = 2 * y + dy
                    for q in range(2):
                        p = ps.tile([128, BC], F32, tag="ps")
                        for kp in range(3):
                            v = q * 3 + kp
                            nc.tensor.matmul(
                                out=p[:80, :],
                                lhsT=w2t8[:, 2 * v:2 * v + 2, :],
                                rhs=h1all[:, oy + 2 * kp:oy + 2 * kp + 2, :],
                                start=(kp == 0), stop=(kp == 2), perf_mode=DR)
                        cps[(dy, q)] = p
                a0 = tmp.tile([80, BC], BF16, tag="mx2")
                nc.scalar.activation(out=a0[:], in_=cps[(0, 0)][:80, :],
                                     func=AF.Relu)
                a1 = tmp.tile([80, BC], BF16, tag="mx2")
                nc.scalar.activation(out=a1[:], in_=cps[(0, 1)][:80, :],
                                     func=AF.Relu)
                b0 = tmp.tile([80, BC], BF16, tag="mx2")
                nc.vector.tensor_max(b0[:], cps[(1, 0)][:80, :], a0[:])
                b1 = tmp.tile([80, BC], BF16, tag="mx2")
                nc.vector.tensor_max(b1[:], cps[(1, 1)][:80, :], a1[:])
                nc.vector.tensor_max(fyall[:, y, :], b0[:], b1[:])

            # ---- stage C: TH_j [128, BC] bf16 (tree 2j @ 0:50, 2j+1 @ 64:114)
            # psum = 512 * z1_true; fused relu rescales by 1/512.
            th = {}
            for j in range(16):
                p = ps.tile([128, BC], F32, tag="ps")
                nc.tensor.matmul(out=p[:], lhsT=w1p8[:, 4 * j:4 * j + 2, :],
                                 rhs=fyall[:, 0:2, :],
                                 start=True, stop=False, perf_mode=DR)
                nc.tensor.matmul(out=p[:], lhsT=w1p8[:, 4 * j + 2:4 * j + 4, :],
                                 rhs=fyall[:, 2:4, :],
                                 start=False, stop=True, perf_mode=DR)
                t = work.tile([128, BC], BF16, tag=f"th_{j}")
                nc.vector.tensor_scalar(out=t[:], in0=p[:],
                                        scalar1=1.0 / 512.0, scalar2=0.0,
                                        op0=AluOpType.mult, op1=AluOpType.max)
                th[j] = t

            # ---- phase D1: all z matmuls + softplus(z) = ln(1+exp(z))
            sall = {}
            for j in range(16):
                pz0 = ps.tile([128, BC], F32, tag="ps")
                pz1 = ps.tile([128, BC], F32, tag="ps")
                for s_, pz in ((0, pz0), (1, pz1)):
                    t_ = 2 * j + s_
                    c0 = t_ * 128
                    r0 = s_ * 64
                    nc.tensor.matmul(out=pz[:],
                                     lhsT=w2all[r0:r0 + 50, c0:c0 + 128],
                                     rhs=th[j][r0:r0 + 50, :],
                                     start=True, stop=True)
                for s_, pz in ((0, pz0), (1, pz1)):
                    t_ = 2 * j + s_
                    e = tmp.tile([128, BC], BF16, tag="e")
                    nc.scalar.activation(out=e[:], in_=pz[:], func=AF.Exp,
                                         bias=0.0, scale=1.0)
                    s = work.tile([128, BC], BF16, tag=f"s_{t_}")
                    nc.scalar.activation(out=s[:], in_=e[:], func=AF.Ln,
                                         bias=1.0, scale=1.0)
                    sall[t_] = s

            # ---- phase D2: logmu = w2a.th - P.s ; mu = exp ; py += lp.mu
            py = pyp.tile([128, BC], F32, tag="py")
            for j in range(16):
                p0 = ps.tile([128, BC], F32, tag="ps")
                p1 = ps.tile([128, BC], F32, tag="ps")
                for s_, p in ((0, p0), (1, p1)):
                    t_ = 2 * j + s_
                    c0 = t_ * 128
                    r0 = s_ * 64
                    nc.tensor.matmul(out=p[:],
                                     lhsT=w2aall[r0:r0 + 50, c0:c0 + 128],
                                     rhs=th[j][r0:r0 + 50, :],
                                     start=True, stop=False)
                for s_, p in ((0, p0), (1, p1)):
                    nc.tensor.matmul(out=p[:], lhsT=negp[:],
                                     rhs=sall[2 * j + s_][:],
                                     start=False, stop=True)
                for s_, p in ((0, p0), (1, p1)):
                    t_ = 2 * j + s_
                    c0 = t_ * 128
                    mu = tmp.tile([128, BC], BF16, tag="mu")
                    nc.scalar.activation(out=mu[:], in_=p[:], func=AF.Exp,
                                         bias=0.0, scale=1.0)
                    nc.tensor.matmul(out=py[:], lhsT=lpall[:, c0:c0 + 128],
                                     rhs=mu[:],
                                     start=(t_ == 0), stop=(t_ == 31),
                                     skip_group_check=True)

            out_t = work.tile([10, BC], F32, tag="out")
            nc.scalar.activation(out=out_t[:], in_=py[:10, :], func=AF.Ln)
            nc.sync.dma_start(out=d_out, in_=out_t[:])

        if n_loop == 1:
            _compute()
        else:
            with tc.For_i(0, n_loop, 1):
                _compute()


_NC_CACHE = None


def _get_nc():
    global _NC_CACHE
    if _NC_CACHE is None:
        _NC_CACHE = _build_nc()
    return _NC_CACHE


def make_in_maps(inputs):
    pre = _precompute(inputs)
    shared = {
        "tq8": pre["tq8"].reshape(112, 16, 128).astype(F8NP),
        "w2t8": pre["w2t8"].reshape(120, 12, 80).astype(F8NP),
        "w1p8": pre["w1p8"].reshape(80, 64, 128).astype(F8NP),
        "w2all": pre["w2all"].astype(BF),
        "w2aall": pre["w2aall"].astype(BF),
        "negp": pre["negp"].astype(BF),
        "lpall": pre["lpall"].astype(BF),
    }
    in_maps = []
    for c in range(NCORES):
        m = dict(shared)
        m["xt"] = np.ascontiguousarray(
            pre["xt"][:, :, c * BC:(c + 1) * BC]).astype(F8NP)
        in_maps.append(m)
    return in_maps


def kernel(**inputs):
    nc = _get_nc()
    in_maps = make_in_maps(inputs)
    res = run_bass_kernel_spmd(nc, in_maps, core_ids=list(range(NCORES)))
    outs = [res.results[c]["out"] for c in range(NCORES)]  # each [10, BC]
    full = np.concatenate(outs, axis=1)  # [10, B]
    return np.ascontiguousarray(full.T).astype(np.float32)  # [B, 10]
